# revision 1
# baseline (speedup 1.0000x reference)
"""BuddingLayer Trainium2 kernel (8-core, routed + fp8, PE/DVE split).

Reference computation (N = size_in = 8192, O = size_out = 8192):
    mask  = (x * saturated) != 0                   # ~half the neurons
    h2    = tiny per-neuron MLP(x)                              [N,3]
    h3    = relu(sum_i W3[n,o,i] * h2[n,i] + b3[n,o])           [N,O]
    u[o]  = sum_n mask[n] * h3[n,o]
    out   = weight @ (x * ~saturated) + bias + u

Host-side routing gathers the ~4112 active experts (mask=1) and ~4080
active dense columns; all big streams are fp8e4 (tolerance 2e-2, |u| ~
643 >> |dense| ~ 0.35).  Per-core stream ~21 MB -> ~60 us DMA floor.

Per-core split of per = ceil(A/8) ~ 514 experts:
  PE path (nslab slabs of 128): c = h2.W3 + b3 via two accumulated
    DoubleRow fp8 matmuls per slab: mm1 contracts pairs (w3_0, w3_1),
    mm2 pairs (w3_2, b3).  The stationaries are diagonal [128, 2, 128]
    tiles built on device from h2 (x8, fp8; the b3 coefficient row is
    the 8*I indicator itself), undone exactly by relu(scale=1/8).
    DoubleRow streams fp8 at 2 elem/cycle/partition.
  DVE path (nsub subtiles of 128): 3x scalar_tensor_tensor over fp8
    plane tiles + relu; keeps DVE busy without over-committing it.
  Tail (the % 128 remainder): o-transposed [o%128, o//128] tiles.
u partials reduce through PSUM via ones-matmuls; the dense matvec is a
DoubleRow fp8 matmul too (weights prescaled x1024 out of the fp8
subnormal range, undone in the bias epilogue).
"""

import sys

import numpy as np

_TRN = "/opt/trn_rl_repo"
if _TRN not in sys.path:
    sys.path.insert(0, _TRN)

import ml_dtypes

import concourse.bacc as bacc
import concourse.mybir as mybir
from concourse import tile
from concourse.bass_utils import run_bass_kernel_spmd

F32 = mybir.dt.float32
BF16 = mybir.dt.bfloat16
FP8 = mybir.dt.float8e4
AF = mybir.ActivationFunctionType
ALU = mybir.AluOpType
AX = mybir.AxisListType
PM = mybir.MatmulPerfMode

NP_BF16 = ml_dtypes.bfloat16
NP_FP8 = ml_dtypes.float8_e4m3

N_CORES = 8
SIZE_IN = 8192
SIZE_OUT = 8192
OC = SIZE_OUT // 128          # o-chunks for the tail layout
WT_SCALE = 1024.0             # dense weights are subnormal in fp8; prescale
H2S = 8.0                     # h2 scale in the PE stationary (undone in relu)


def build_program(
    nsub,                      # 128-expert DVE-path subtiles per core
    nslab,                     # 128-expert PE-path DoubleRow slabs per core
    n_tail,                    # leftover experts per core (o-transposed path)
    nkc2,                      # 256-row DoubleRow chunks for dense
    size_out=SIZE_OUT,
    n_cores=N_CORES,
    o_blk=512,
    w3_bufs=8,
    b3_bufs=8,
    pe_bufs=16,
    wt_bufs=6,
    stk_bufs=1,
    upsum_bufs=2,
    ksteps_frac=0.75,
    enable_asserts=False,
):
    m_own = size_out // n_cores
    nexp = nsub + nslab        # 128-expert groups with standard h2 layout
    n_v = 128 * nsub

    nc = bacc.Bacc(
        "TRN2",
        target_bir_lowering=False,
        debug=False,
        enable_asserts=enable_asserts,
        num_devices=n_cores,
    )

    d = {}
    d["x_own"] = nc.dram_tensor("x_own", [128, nexp], F32, kind="ExternalInput")
    d["w1"] = nc.dram_tensor("w1", [128, nexp, 3, 3], F32, kind="ExternalInput")
    d["b1"] = nc.dram_tensor("b1", [128, nexp, 3], F32, kind="ExternalInput")
    d["w2"] = nc.dram_tensor("w2", [128, nexp, 3, 3], F32, kind="ExternalInput")
    d["b2"] = nc.dram_tensor("b2", [128, nexp, 3], F32, kind="ExternalInput")
    if nsub:
        d["w3"] = nc.dram_tensor("w3", [n_v, 3, size_out], FP8, kind="ExternalInput")
        d["b3"] = nc.dram_tensor("b3", [n_v, size_out], FP8, kind="ExternalInput")
    if nslab:
        # slab s, partition k = expert: wpa pairs (w3_0, w3_1), wpb (w3_2, b3)
        d["wpa"] = nc.dram_tensor("wpa", [nslab * 128, 2, size_out], FP8,
                                  kind="ExternalInput")
        d["wpb"] = nc.dram_tensor("wpb", [nslab * 128, 2, size_out], FP8,
                                  kind="ExternalInput")
        d["cind"] = nc.dram_tensor("cind", [128, 128], FP8, kind="ExternalInput")
    d["xg"] = nc.dram_tensor("xg", [128, 2, nkc2], FP8, kind="ExternalInput")
    d["wt"] = nc.dram_tensor("wt", [nkc2 * 128, 2, m_own], FP8, kind="ExternalInput")
    d["bias"] = nc.dram_tensor("bias", [1, m_own], F32, kind="ExternalInput")
    if n_tail:
        d["xt"] = nc.dram_tensor("xt", [128, n_tail], F32, kind="ExternalInput")
        d["w1t"] = nc.dram_tensor("w1t", [128, n_tail, 3, 3], F32, kind="ExternalInput")
        d["b1t"] = nc.dram_tensor("b1t", [128, n_tail, 3], F32, kind="ExternalInput")
        d["w2t"] = nc.dram_tensor("w2t", [128, n_tail, 3, 3], F32, kind="ExternalInput")
        d["b2t"] = nc.dram_tensor("b2t", [128, n_tail, 3], F32, kind="ExternalInput")
        d["w3t"] = nc.dram_tensor("w3t", [128, n_tail, 3, OC], BF16, kind="ExternalInput")
        d["b3t"] = nc.dram_tensor("b3t", [128, n_tail, OC], BF16, kind="ExternalInput")
        d["ut_out"] = nc.dram_tensor("ut_out", [128, OC], F32, kind="ExternalOutput")
    d["u_out"] = nc.dram_tensor("u_out", [1, size_out], F32, kind="ExternalOutput")
    d["dense_out"] = nc.dram_tensor("dense_out", [1, m_own], F32, kind="ExternalOutput")

    def small_mlp(cp, x_sb, w1, b1, w2, b2, nt, pfx):
        h0 = cp.tile([128, nt], F32, tag=f"{pfx}h0")
        nc.vector.tensor_scalar_mul(h0[:], x_sb[:], 1.0 / 3.0)
        rs1 = cp.tile([128, nt, 3], F32, tag=f"{pfx}rs1")
        nc.vector.tensor_reduce(rs1[:], w1[:], axis=AX.X, op=ALU.add)
        h1 = cp.tile([128, nt, 3], F32, tag=f"{pfx}h1")
        for t in range(nt):
            nc.vector.scalar_tensor_tensor(
                h1[:, t, :], rs1[:, t, :], h0[:, t : t + 1], b1[:, t, :],
                op0=ALU.mult, op1=ALU.add,
            )
        nc.vector.tensor_scalar_max(h1[:], h1[:], 0.0)
        h2 = cp.tile([128, nt, 3], F32, tag=f"{pfx}h2")
        for t in range(nt):
            nc.vector.scalar_tensor_tensor(
                h2[:, t, :], w2[:, t, :, 0], h1[:, t, 0:1], b2[:, t, :],
                op0=ALU.mult, op1=ALU.add,
            )
            for i in (1, 2):
                nc.vector.scalar_tensor_tensor(
                    h2[:, t, :], w2[:, t, :, i], h1[:, t, i : i + 1], h2[:, t, :],
                    op0=ALU.mult, op1=ALU.add,
                )
        nc.vector.tensor_scalar_max(h2[:], h2[:], 0.0)
        return h2

    with tile.TileContext(nc) as tc:
        with (
            tc.tile_pool(name="const", bufs=1) as cp,
            tc.tile_pool(name="w3p", bufs=w3_bufs) as w3p,
            tc.tile_pool(name="b3p", bufs=b3_bufs) as b3p,
            tc.tile_pool(name="pep", bufs=pe_bufs) as pep,
            tc.tile_pool(name="wtp", bufs=wt_bufs) as wtp,
            tc.tile_pool(name="accp", bufs=2) as accp,
            tc.tile_pool(name="rp", bufs=3) as rp,
            tc.tile_pool(name="outp", bufs=2) as outp,
            tc.tile_pool(name="pp", bufs=1, space="PSUM") as pp,
        ):
            # ---- small constant loads -------------------------------------
            x_own = cp.tile([128, nexp], F32)
            nc.gpsimd.dma_start(x_own[:], d["x_own"][:])
            w1 = cp.tile([128, nexp, 3, 3], F32)
            nc.gpsimd.dma_start(w1[:], d["w1"][:])
            b1 = cp.tile([128, nexp, 3], F32)
            nc.gpsimd.dma_start(b1[:], d["b1"][:])
            w2 = cp.tile([128, nexp, 3, 3], F32)
            nc.gpsimd.dma_start(w2[:], d["w2"][:])
            b2 = cp.tile([128, nexp, 3], F32)
            nc.gpsimd.dma_start(b2[:], d["b2"][:])
            if nslab:
                cind = cp.tile([128, 128], FP8)
                nc.gpsimd.dma_start(cind[:], d["cind"][:])
            xg = cp.tile([128, 2, nkc2], FP8)
            nc.gpsimd.dma_start(xg[:], d["xg"][:])
            bias_sb = cp.tile([1, m_own], F32)
            nc.gpsimd.dma_start(bias_sb[:], d["bias"][:])
            if n_tail:
                xt = cp.tile([128, n_tail], F32)
                nc.gpsimd.dma_start(xt[:], d["xt"][:])
                w1t = cp.tile([128, n_tail, 3, 3], F32)
                nc.gpsimd.dma_start(w1t[:], d["w1t"][:])
                b1t = cp.tile([128, n_tail, 3], F32)
                nc.gpsimd.dma_start(b1t[:], d["b1t"][:])
                w2t = cp.tile([128, n_tail, 3, 3], F32)
                nc.gpsimd.dma_start(w2t[:], d["w2t"][:])
                b2t = cp.tile([128, n_tail, 3], F32)
                nc.gpsimd.dma_start(b2t[:], d["b2t"][:])
                w3t = cp.tile([128, n_tail, 3, OC], BF16)
                nc.gpsimd.dma_start(w3t[:], d["w3t"][:])
                b3t_sb = cp.tile([128, n_tail, OC], BF16)
                nc.gpsimd.dma_start(b3t_sb[:], d["b3t"][:])

            ones_bf = cp.tile([128, 1], BF16)
            nc.vector.memset(ones_bf[:], 1.0)

            # ---- h2 for all expert paths ----------------------------------
            h2 = small_mlp(cp, x_own, w1, b1, w2, b2, nexp, "v")
            if n_tail:
                h2t = small_mlp(cp, xt, w1t, b1t, w2t, b2t, n_tail, "t")
                ut = cp.tile([128, OC], F32)
            if nslab:
                # diagonal stationaries: Sa = (8I)*h2_0 | (8I)*h2_1,
                # Sb = (8I)*h2_2 | 8I   (bias row coefficient)
                stat = []
                for s in range(nslab):
                    t = nsub + s
                    Sa = cp.tile([128, 2, 128], FP8, tag=f"Sa{s}")
                    Sb = cp.tile([128, 2, 128], FP8, tag=f"Sb{s}")
                    for c in (0, 1):
                        nc.vector.tensor_scalar(
                            Sa[:, c, :], cind[:], h2[:, t, c : c + 1], None,
                            op0=ALU.mult,
                        )
                    nc.vector.tensor_scalar(
                        Sb[:, 0, :], cind[:], h2[:, t, 2:3], None, op0=ALU.mult,
                    )
                    nc.vector.tensor_copy(Sb[:, 1, :], cind[:])
                    stat.append((Sa, Sb))

            # ---- main streamed loop ---------------------------------------
            o_blocks = []
            o_pos = 0
            while o_pos < size_out:
                rem = size_out - o_pos
                if rem > o_blk:
                    o_blocks.append((o_pos, o_blk))
                    o_pos += o_blk
                elif rem == o_blk and o_blk >= 512:
                    for ln in (o_blk // 2, o_blk // 4, o_blk // 8, o_blk // 8):
                        o_blocks.append((o_pos, ln))
                        o_pos += ln
                else:
                    o_blocks.append((o_pos, rem))
                    o_pos += rem
            steps = len(o_blocks)
            ksteps = max(1, int(steps * ksteps_frac))

            d_psum = pp.tile([1, m_own], F32, tag="dpsum")
            for ob, (o0, o_len) in enumerate(o_blocks):
                u_psum = pp.tile([1, o_len], F32, tag="upsum", bufs=upsum_bufs)
                pre = ob == 0
                # ---------- DMAs for this block ----------
                if nslab:
                    petiles = []
                    for s in range(nslab):
                        ea = nc.gpsimd if (pre and s <= 1) else (
                            nc.sync if s % 2 == 0 else nc.scalar)
                        eb = nc.gpsimd if (pre and s <= 1) else (
                            nc.scalar if s % 2 == 0 else nc.sync)
                        wa = pep.tile([128, 2, o_len], FP8, tag="wpa")
                        ea.dma_start(
                            wa[:],
                            d["wpa"][s * 128 : (s + 1) * 128, :, o0 : o0 + o_len],
                        )
                        wb = pep.tile([128, 2, o_len], FP8, tag="wpb")
                        eb.dma_start(
                            wb[:],
                            d["wpb"][s * 128 : (s + 1) * 128, :, o0 : o0 + o_len],
                        )
                        petiles.append((wa, wb))
                if nsub:
                    vtiles = []
                    for t in range(nsub):
                        w3_eng = nc.gpsimd if (pre and t < 1) else nc.sync
                        b3_eng = nc.gpsimd
                        w3tile = w3p.tile([128, 3, o_len], FP8, tag="w3t")
                        w3_eng.dma_start(
                            w3tile[:],
                            d["w3"][t * 128 : (t + 1) * 128, :, o0 : o0 + o_len],
                        )
                        b3tile = b3p.tile([128, o_len], FP8, tag="b3t")
                        b3_eng.dma_start(
                            b3tile[:],
                            d["b3"][t * 128 : (t + 1) * 128, o0 : o0 + o_len],
                        )
                        vtiles.append((w3tile, b3tile))

                n_chunks = (o_len + 511) // 512
                # ---------- PE path: DoubleRow c-matmuls + relu + reduce ----
                if nslab:
                    for j in range(n_chunks):
                        lo = j * 512
                        hi = min(lo + 512, o_len)
                        f = hi - lo
                        stks = []
                        # all c-matmuls first, then relu+reduce, so the
                        # reduce stationary (ones) is loaded once per chunk
                        for s in range(nslab):
                            wa, wb = petiles[s]
                            Sa, Sb = stat[s]
                            stk = pp.tile([128, 512], F32, tag=f"stk{s}",
                                          bufs=stk_bufs)
                            nc.tensor.matmul(
                                stk[:, 0:f], Sa[:], wa[:, :, lo:hi],
                                start=True, stop=False, perf_mode=PM.DoubleRow,
                            )
                            nc.tensor.matmul(
                                stk[:, 0:f], Sb[:], wb[:, :, lo:hi],
                                start=False, stop=True, perf_mode=PM.DoubleRow,
                            )
                            stks.append(stk)
                        for s in range(nslab):
                            rpe = rp.tile([128, 512], BF16, tag=f"rpe{s}")
                            nc.scalar.activation(
                                rpe[:, 0:f], stks[s][:, 0:f], AF.Relu,
                                scale=1.0 / H2S,
                            )
                            # start/stop are per 512-col psum region
                            nc.tensor.matmul(
                                u_psum[0:1, lo:hi],
                                ones_bf[:],
                                rpe[:, 0:f],
                                start=(s == 0),
                                stop=(not nsub) and s == nslab - 1,
                            )
                # ---------- DVE path: stt chain + relu + reduce ----
                if nsub:
                    for t in range(nsub):
                        w3tile, b3tile = vtiles[t]
                        acc = accp.tile([128, o_len], BF16, tag="acc")
                        nc.vector.scalar_tensor_tensor(
                            acc[:], w3tile[:, 0, :], h2[:, t, 0:1], b3tile[:],
                            op0=ALU.mult, op1=ALU.add,
                        )
                        for i in (1, 2):
                            nc.vector.scalar_tensor_tensor(
                                acc[:], w3tile[:, i, :], h2[:, t, i : i + 1], acc[:],
                                op0=ALU.mult, op1=ALU.add,
                            )
                        r = rp.tile([128, o_len], BF16, tag="r")
                        nc.vector.tensor_scalar_max(r[:], acc[:], 0.0)
                        for j in range(n_chunks):
                            lo, hi = j * 512, min((j + 1) * 512, o_len)
                            nc.tensor.matmul(
                                u_psum[0:1, lo:hi],
                                ones_bf[:],
                                r[:, lo:hi],
                                start=(not nslab) and t == 0,
                                stop=(t == nsub - 1),
                            )
                # ---------- dense matvec (DoubleRow), front-loaded ----------
                if ob < ksteps:
                    for kc in range(ob * nkc2 // ksteps, (ob + 1) * nkc2 // ksteps):
                        wtt = wtp.tile([128, 2, m_own], FP8, tag="wtt")
                        nc.gpsimd.dma_start(
                            wtt[:], d["wt"][kc * 128 : (kc + 1) * 128, :, :]
                        )
                        for mb in range((m_own + 511) // 512):
                            lo, hi = mb * 512, min((mb + 1) * 512, m_own)
                            nc.tensor.matmul(
                                d_psum[0:1, lo:hi],
                                xg[:, :, kc : kc + 1],
                                wtt[:, :, lo:hi],
                                start=(kc == 0), stop=(kc == nkc2 - 1),
                                perf_mode=PM.DoubleRow,
                            )
                    if ob == ksteps - 1:
                        dense_sb = outp.tile([1, m_own], F32, tag="dense_sb")
                        nc.vector.scalar_tensor_tensor(
                            dense_sb[:], d_psum[:], 1.0 / WT_SCALE, bias_sb[:],
                            op0=ALU.mult, op1=ALU.add,
                        )
                        nc.gpsimd.dma_start(d["dense_out"][:], dense_sb[:])
                # ---------- tail experts, once, early ----------
                if n_tail and ob == 1:
                    for e in range(n_tail):
                        tacc = accp.tile([128, OC], BF16, tag=f"tacc{e}")
                        nc.vector.scalar_tensor_tensor(
                            tacc[:], w3t[:, e, 0, :], h2t[:, e, 0:1],
                            b3t_sb[:, e, :], op0=ALU.mult, op1=ALU.add,
                        )
                        for i in (1, 2):
                            nc.vector.scalar_tensor_tensor(
                                tacc[:], w3t[:, e, i, :], h2t[:, e, i : i + 1],
                                tacc[:], op0=ALU.mult, op1=ALU.add,
                            )
                        if e == 0:
                            nc.scalar.activation(ut[:], tacc[:], AF.Relu)
                        else:
                            rt = rp.tile([128, OC], F32, tag="rt")
                            nc.scalar.activation(rt[:], tacc[:], AF.Relu)
                            nc.vector.tensor_tensor(ut[:], ut[:], rt[:], op=ALU.add)
                    nc.gpsimd.dma_start(d["ut_out"][:], ut[:])
                u_sb = outp.tile([1, o_len], F32, tag="u_sb")
                nc.vector.tensor_copy(u_sb[:], u_psum[:])
                nc.sync.dma_start(d["u_out"][0:1, o0 : o0 + o_len], u_sb[:])

    nc.compile()
    return nc, d


def route(inputs):
    """Host-side routing: active experts + active dense columns."""
    x = np.asarray(inputs["x"], dtype=np.float32)
    sat = np.asarray(inputs["saturated"]).astype(bool)
    act = np.nonzero(sat & (x != 0))[0]
    dcols = np.nonzero(~sat)[0]
    per = -(-len(act) // N_CORES)            # ceil
    ngrp = per // 128                        # full 128-expert groups
    nsub = max(0, min(1, ngrp)) if ngrp <= 5 else ngrp - 4
    nslab = ngrp - nsub                      # PSUM: nslab + 4 banks <= 8
    n_tail = per - 128 * ngrp
    nkc2 = -(-len(dcols) // 256)
    return act, dcols, per, nsub, nslab, n_tail, nkc2


def make_in_maps(inputs, act, dcols, per, nsub, nslab, n_tail, nkc2):
    x = np.asarray(inputs["x"], dtype=np.float32)
    weight = np.asarray(inputs["weight"], dtype=np.float32)
    bias = np.asarray(inputs["bias"], dtype=np.float32)
    W1 = np.asarray(inputs["W1"], dtype=np.float32)
    b1 = np.asarray(inputs["b1"], dtype=np.float32)
    W2 = np.asarray(inputs["W2"], dtype=np.float32)
    b2 = np.asarray(inputs["b2"], dtype=np.float32)
    W3 = np.asarray(inputs["W3"], dtype=np.float32)
    b3 = np.asarray(inputs["b3"], dtype=np.float32)

    m_own = SIZE_OUT // N_CORES
    nexp = nsub + nslab
    n_v = 128 * nsub
    n_pe = 128 * nslab
    Dp = nkc2 * 256

    xg_full = np.zeros(Dp, dtype=np.float32)
    xg_full[: len(dcols)] = x[dcols]
    # DoubleRow pairs: partition p of chunk kc holds rows kc*256+2p, +1
    xg = np.ascontiguousarray(
        xg_full.reshape(nkc2, 128, 2).transpose(1, 2, 0)
    ).astype(NP_FP8)

    cind = (H2S * np.eye(128, dtype=np.float32)).astype(NP_FP8)

    in_maps = []
    for i in range(N_CORES):
        ids = act[i * per : (i + 1) * per]
        n_live = len(ids)
        if n_live < per:
            ids = np.concatenate([ids, np.zeros(per - n_live, dtype=ids.dtype)])
        gids = ids[: 128 * nexp]             # V-path then PE-path groups
        tids = ids[128 * nexp :]

        m = {
            "bias": bias[i * m_own : (i + 1) * m_own].reshape(1, m_own),
            "xg": xg,
            "x_own": np.ascontiguousarray(x[gids].reshape(nexp, 128).T),
            "w1": np.ascontiguousarray(
                W1[gids].reshape(nexp, 128, 3, 3).transpose(1, 0, 2, 3)),
            "b1": np.ascontiguousarray(
                b1[gids].reshape(nexp, 128, 3).transpose(1, 0, 2)),
            "w2": np.ascontiguousarray(
                W2[gids].reshape(nexp, 128, 3, 3).transpose(1, 0, 2, 3)),
            "b2": np.ascontiguousarray(
                b2[gids].reshape(nexp, 128, 3).transpose(1, 0, 2)),
        }

        slm = slice(i * m_own, (i + 1) * m_own)
        wtg = np.zeros((Dp, m_own), dtype=np.float32)
        wtg[: len(dcols)] = weight[slm][:, dcols].T
        m["wt"] = np.ascontiguousarray(
            (wtg * WT_SCALE).reshape(nkc2 * 128, 2, m_own)
        ).astype(NP_FP8)

        if nsub:
            vids = gids[:n_v]
            w3v = np.ascontiguousarray(W3[vids].transpose(0, 2, 1)).astype(NP_FP8)
            b3v = b3[vids].astype(NP_FP8)
            live_v = min(max(n_live, 0), n_v)
            if live_v < n_v:
                w3v[live_v:] = 0
                b3v[live_v:] = 0
            m.update({"w3": w3v, "b3": b3v})
        if nslab:
            pids = gids[n_v:]
            W3p = W3[pids].astype(NP_FP8)                    # [n_pe, O, 3]
            b3p8 = b3[pids].astype(NP_FP8)
            live_p = min(max(n_live - n_v, 0), n_pe)
            if live_p < n_pe:
                W3p[live_p:] = 0
                b3p8[live_p:] = 0
            wpa = np.empty((n_pe, 2, SIZE_OUT), dtype=NP_FP8)
            wpb = np.empty((n_pe, 2, SIZE_OUT), dtype=NP_FP8)
            wpa[:, 0, :] = W3p[:, :, 0]
            wpa[:, 1, :] = W3p[:, :, 1]
            wpb[:, 0, :] = W3p[:, :, 2]
            wpb[:, 1, :] = b3p8
            m.update({"wpa": wpa, "wpb": wpb, "cind": cind})
        if n_tail:
            nt_live = max(0, min(n_tail, n_live - 128 * nexp))
            w3tt = np.ascontiguousarray(
                W3[tids]
                .transpose(0, 2, 1)
                .reshape(n_tail, 3, OC, 128)
                .transpose(3, 0, 1, 2)
            ).astype(NP_BF16)
            b3tt = np.ascontiguousarray(
                b3[tids].reshape(n_tail, OC, 128).transpose(2, 0, 1)
            ).astype(NP_BF16)
            if nt_live < n_tail:
                w3tt[:, nt_live:] = 0
                b3tt[:, nt_live:] = 0
            m.update({
                "xt": np.ascontiguousarray(np.broadcast_to(x[tids], (128, n_tail))),
                "w1t": np.ascontiguousarray(
                    np.broadcast_to(W1[tids], (128, n_tail, 3, 3))),
                "b1t": np.ascontiguousarray(
                    np.broadcast_to(b1[tids], (128, n_tail, 3))),
                "w2t": np.ascontiguousarray(
                    np.broadcast_to(W2[tids], (128, n_tail, 3, 3))),
                "b2t": np.ascontiguousarray(
                    np.broadcast_to(b2[tids], (128, n_tail, 3))),
                "w3t": w3tt,
                "b3t": b3tt,
            })
        in_maps.append(m)
    return in_maps


def combine_outputs(results, names, n_tail):
    u = np.zeros(SIZE_OUT, dtype=np.float64)
    dense = []
    for res in results:
        u += res[names["u_out"].name].reshape(-1).astype(np.float64)
        if n_tail:
            ut = res[names["ut_out"].name].astype(np.float64)  # [128, OC]
            u += ut.T.reshape(-1)                              # o = c*128 + p
        dense.append(res[names["dense_out"].name].reshape(-1))
    out = np.concatenate(dense).astype(np.float64) + u
    return out.astype(np.float32)


_CACHE = {}
CONFIG = {}


def _get_program(nsub, nslab, n_tail, nkc2):
    key = (nsub, nslab, n_tail, nkc2, tuple(sorted(CONFIG.items())))
    if key not in _CACHE:
        _CACHE[key] = build_program(nsub, nslab, n_tail, nkc2, **CONFIG)
    return _CACHE[key]


def kernel(**inputs):
    act, dcols, per, nsub, nslab, n_tail, nkc2 = route(inputs)
    nc, names = _get_program(nsub, nslab, n_tail, nkc2)
    in_maps = make_in_maps(inputs, act, dcols, per, nsub, nslab, n_tail, nkc2)
    keyed = [{names[k].name: v for k, v in m.items()} for m in in_maps]
    res = run_bass_kernel_spmd(nc, keyed, core_ids=list(range(N_CORES)))
    return combine_outputs(res.results, names, n_tail)



# revision 2
# speedup vs baseline: 1.1699x; 1.1699x over previous
"""BuddingLayer Trainium2 kernel (8-core, routed + fp8, contiguous-stream).

Reference computation (N = size_in = 8192, O = size_out = 8192):
    mask  = (x * saturated) != 0                   # ~half the neurons
    h2    = tiny per-neuron MLP(x)                              [N,3]
    h3    = relu(sum_i W3[n,o,i] * h2[n,i] + b3[n,o])           [N,O]
    u[o]  = sum_n mask[n] * h3[n,o]
    out   = weight @ (x * ~saturated) + bias + u

Host-side routing gathers the ~4112 active experts (mask=1) and ~4080
active dense columns; all big streams are fp8e4 (tolerance 2e-2).
Per-core stream ~21 MB -> ~60 us DMA floor at 358 GB/s.

v2 design (from 113.5us baseline profile):
  * The baseline spent ~130us of aggregate issue-engine time on ~200
    small strided DMAs (512B packets) and ~25us of Tensor time on bf16
    ones-reduce matmuls.  Both are restructured away:
  * Expert stream is pre-tiled on host into per-o-block CONTIGUOUS
    super-tiles pe[b] = [128, ns, 2(pair), 2(row), 512] fp8 -> ONE
    1 MB DMA per block (16 total), 8 KB/partition contiguous.
  * All ns=4 slabs go through the PE DoubleRow path (2 matmuls/slab
    into stk_s).  relu is a 2-op chain per psum tile pair: Scalar
    ACTIVATE(relu) writes row r of a [128,2,512] fp8 acc tile for slab
    2r, DVE scalar_tensor_tensor max-add accumulates slab 2r+1.
  * One DoubleRow reduce matmul per block contracts acc against a
    0.125-valued block-column selector -> row b of a single [16,512]
    psum bank accumulated across all 16 blocks; one copy + one store
    at the end.  (0.125 = 1/H2S undoes the stationary's 8x scale.)
  * Dense matvec: wt pre-tiled into 8 contiguous pair-tiles
    [128, 2(kc), 2(row), 1024] fp8, DoubleRow with xg stationary.
  * Small MLP consts for the 4 slab groups + 2 tail experts are merged
    (nt=6) into single per-field DMAs.
"""

import sys

import numpy as np

_TRN = "/opt/trn_rl_repo"
if _TRN not in sys.path:
    sys.path.insert(0, _TRN)

import ml_dtypes

import concourse.bacc as bacc
import concourse.mybir as mybir
from concourse import tile
from concourse.bass_utils import run_bass_kernel_spmd

F32 = mybir.dt.float32
BF16 = mybir.dt.bfloat16
FP8 = mybir.dt.float8e4
AF = mybir.ActivationFunctionType
ALU = mybir.AluOpType
AX = mybir.AxisListType
PM = mybir.MatmulPerfMode

NP_BF16 = ml_dtypes.bfloat16
NP_FP8 = ml_dtypes.float8_e4m3

N_CORES = 8
SIZE_IN = 8192
SIZE_OUT = 8192
OC = SIZE_OUT // 128          # o-chunks for the tail layout
O_BLK = 512                   # o-block (one psum bank of f32)
WT_SCALE = 1024.0             # dense weights are subnormal in fp8; prescale
H2S = 8.0                     # h2 scale in the PE stationary (undone in reduce)


def build_program(
    ns,                        # 128-expert PE slabs per core (must be even)
    n_tail,                    # leftover experts per core (o-transposed path)
    nkc2,                      # 256-row DoubleRow chunks for dense (even)
    size_out=SIZE_OUT,
    n_cores=N_CORES,
    pe_bufs=6,
    wt_bufs=3,
    acc_bufs=3,
    tail_blk=2,
    enable_asserts=False,
):
    assert ns % 2 == 0 and ns >= 2
    assert nkc2 % 2 == 0
    m_own = size_out // n_cores
    NB = size_out // O_BLK
    npair = nkc2 // 2
    nrow = ns // 2
    nt_all = ns + n_tail       # merged small-MLP groups (slab + tail)

    nc = bacc.Bacc(
        "TRN2",
        target_bir_lowering=False,
        debug=False,
        enable_asserts=enable_asserts,
        num_devices=n_cores,
    )

    d = {}
    d["pe"] = nc.dram_tensor("pe", [NB, 128, ns, 2, 2, O_BLK], FP8,
                             kind="ExternalInput")
    d["wt"] = nc.dram_tensor("wt", [npair, 128, 2, 2, m_own], FP8,
                             kind="ExternalInput")
    d["xg"] = nc.dram_tensor("xg", [128, 2, nkc2], FP8, kind="ExternalInput")
    d["cind"] = nc.dram_tensor("cind", [128, 128], FP8, kind="ExternalInput")
    d["xall"] = nc.dram_tensor("xall", [128, nt_all], F32, kind="ExternalInput")
    d["w1"] = nc.dram_tensor("w1", [128, nt_all, 3, 3], F32, kind="ExternalInput")
    d["b1"] = nc.dram_tensor("b1", [128, nt_all, 3], F32, kind="ExternalInput")
    d["w2"] = nc.dram_tensor("w2", [128, nt_all, 3, 3], F32, kind="ExternalInput")
    d["b2"] = nc.dram_tensor("b2", [128, nt_all, 3], F32, kind="ExternalInput")
    d["bias"] = nc.dram_tensor("bias", [1, m_own], F32, kind="ExternalInput")
    if n_tail:
        d["tbf"] = nc.dram_tensor("tbf", [128, n_tail, 4, OC], BF16,
                                  kind="ExternalInput")
        d["ut_out"] = nc.dram_tensor("ut_out", [128, OC], F32,
                                     kind="ExternalOutput")
    d["u_out"] = nc.dram_tensor("u_out", [NB, O_BLK], F32, kind="ExternalOutput")
    d["dense_out"] = nc.dram_tensor("dense_out", [1, m_own], F32,
                                    kind="ExternalOutput")

    def small_mlp(cp, x_sb, w1, b1, w2, b2, nt, pfx):
        h0 = cp.tile([128, nt], F32, tag=f"{pfx}h0")
        nc.vector.tensor_scalar_mul(h0[:], x_sb[:], 1.0 / 3.0)
        rs1 = cp.tile([128, nt, 3], F32, tag=f"{pfx}rs1")
        nc.vector.tensor_reduce(rs1[:], w1[:], axis=AX.X, op=ALU.add)
        h1 = cp.tile([128, nt, 3], F32, tag=f"{pfx}h1")
        for t in range(nt):
            nc.vector.scalar_tensor_tensor(
                h1[:, t, :], rs1[:, t, :], h0[:, t : t + 1], b1[:, t, :],
                op0=ALU.mult, op1=ALU.add,
            )
        nc.vector.tensor_scalar_max(h1[:], h1[:], 0.0)
        h2 = cp.tile([128, nt, 3], F32, tag=f"{pfx}h2")
        for t in range(nt):
            nc.vector.scalar_tensor_tensor(
                h2[:, t, :], w2[:, t, :, 0], h1[:, t, 0:1], b2[:, t, :],
                op0=ALU.mult, op1=ALU.add,
            )
            for i in (1, 2):
                nc.vector.scalar_tensor_tensor(
                    h2[:, t, :], w2[:, t, :, i], h1[:, t, i : i + 1], h2[:, t, :],
                    op0=ALU.mult, op1=ALU.add,
                )
        nc.vector.tensor_scalar_max(h2[:], h2[:], 0.0)
        return h2

    with tile.TileContext(nc) as tc:
        with (
            tc.tile_pool(name="const", bufs=1) as cp,
            tc.tile_pool(name="pep", bufs=pe_bufs) as pep,
            tc.tile_pool(name="wtp", bufs=wt_bufs) as wtp,
            tc.tile_pool(name="accp", bufs=acc_bufs) as accp,
            tc.tile_pool(name="rp", bufs=2) as rp,
            tc.tile_pool(name="outp", bufs=2) as outp,
            tc.tile_pool(name="pp", bufs=1, space="PSUM") as pp,
        ):
            # ---- const loads (all on gpsimd queue, priority order) --------
            xg = cp.tile([128, 2, nkc2], FP8)
            nc.gpsimd.dma_start(xg[:], d["xg"][:])
            # wt pair 0 early so dense matmuls can fill the warm-up
            wtt0 = wtp.tile([128, 2, 2, m_own], FP8, tag="wt")
            nc.gpsimd.dma_start(wtt0[:], d["wt"][0:1])
            x_all = cp.tile([128, nt_all], F32)
            nc.gpsimd.dma_start(x_all[:], d["xall"][:])
            w1 = cp.tile([128, nt_all, 3, 3], F32)
            nc.gpsimd.dma_start(w1[:], d["w1"][:])
            b1 = cp.tile([128, nt_all, 3], F32)
            nc.gpsimd.dma_start(b1[:], d["b1"][:])
            w2 = cp.tile([128, nt_all, 3, 3], F32)
            nc.gpsimd.dma_start(w2[:], d["w2"][:])
            b2 = cp.tile([128, nt_all, 3], F32)
            nc.gpsimd.dma_start(b2[:], d["b2"][:])
            cind = cp.tile([128, 128], FP8)
            nc.gpsimd.dma_start(cind[:], d["cind"][:])
            bias_sb = cp.tile([1, m_own], F32)
            nc.gpsimd.dma_start(bias_sb[:], d["bias"][:])
            if n_tail:
                tbf = cp.tile([128, n_tail, 4, OC], BF16)
                nc.gpsimd.dma_start(tbf[:], d["tbf"][:])
                ut = cp.tile([128, OC], F32)

            # ---- reduce selector stationaries (device-built) --------------
            # sel[:, b, r, j] = 0.125 iff j == b : directs block b's 2-row
            # relu-acc contraction into row b of the u psum bank.
            sel = cp.tile([128, NB, nrow, NB], FP8)
            nc.vector.memset(sel[:], 0.0)
            for b in range(NB):
                nc.vector.memset(sel[:, b, :, b : b + 1], 1.0 / H2S)

            # ---- h2 for all experts (slab groups + tail groups) -----------
            h2 = small_mlp(cp, x_all, w1, b1, w2, b2, nt_all, "v")

            # diagonal stationaries: Sa = (8I)*h2_0 | (8I)*h2_1,
            # Sb = (8I)*h2_2 | 8I   (bias row coefficient)
            stat = []
            for s in range(ns):
                Sa = cp.tile([128, 2, 128], FP8, tag=f"Sa{s}")
                Sb = cp.tile([128, 2, 128], FP8, tag=f"Sb{s}")
                for c in (0, 1):
                    nc.vector.tensor_scalar(
                        Sa[:, c, :], cind[:], h2[:, s, c : c + 1], None,
                        op0=ALU.mult,
                    )
                nc.vector.tensor_scalar(
                    Sb[:, 0, :], cind[:], h2[:, s, 2:3], None, op0=ALU.mult,
                )
                nc.vector.tensor_copy(Sb[:, 1, :], cind[:])
                stat.append((Sa, Sb))

            # ---- persistent psum tiles ------------------------------------
            u_all = pp.tile([NB, O_BLK], F32, tag="uall")
            d_psum = pp.tile([1, m_own], F32, tag="dpsum")

            # ---- main streamed loop ---------------------------------------
            pend = None            # (block, acc) awaiting its reduce matmul
            for b in range(NB):
                # ---------- one contiguous DMA for the whole block ----------
                pet = pep.tile([128, ns, 2, 2, O_BLK], FP8, tag="pe")
                eng = nc.sync if b % 2 == 0 else nc.scalar
                eng.dma_start(pet[:], d["pe"][b : b + 1])

                # ---------- PE path: 2 DoubleRow matmuls per slab -----------
                stks = []
                for s in range(ns):
                    stk = pp.tile([128, O_BLK], F32, tag=f"stk{s}")
                    Sa, Sb = stat[s]
                    nc.tensor.matmul(
                        stk[:], Sa[:], pet[:, s, 0, :, :],
                        start=True, stop=False, perf_mode=PM.DoubleRow,
                    )
                    nc.tensor.matmul(
                        stk[:], Sb[:], pet[:, s, 1, :, :],
                        start=False, stop=True, perf_mode=PM.DoubleRow,
                    )
                    stks.append(stk)

                # ---------- dense matvec pair-tile (DoubleRow) --------------
                if b < npair:
                    if b == 0:
                        wtt = wtt0
                    else:
                        wtt = wtp.tile([128, 2, 2, m_own], FP8, tag="wt")
                        nc.gpsimd.dma_start(wtt[:], d["wt"][b : b + 1])
                    for q in (0, 1):
                        kc = 2 * b + q
                        for mb in range(m_own // 512):
                            lo, hi = mb * 512, (mb + 1) * 512
                            nc.tensor.matmul(
                                d_psum[0:1, lo:hi],
                                xg[:, :, kc : kc + 1],
                                wtt[:, q, :, lo:hi],
                                start=(kc == 0), stop=(kc == nkc2 - 1),
                                perf_mode=PM.DoubleRow,
                            )
                    if b == npair - 1:
                        dense_sb = outp.tile([1, m_own], F32, tag="dense_sb")
                        nc.vector.scalar_tensor_tensor(
                            dense_sb[:], d_psum[:], 1.0 / WT_SCALE, bias_sb[:],
                            op0=ALU.mult, op1=ALU.add,
                        )
                        nc.gpsimd.dma_start(d["dense_out"][:], dense_sb[:])

                # ---------- deferred reduce for the previous block ----------
                if pend is not None:
                    pb, pacc = pend
                    nc.tensor.matmul(
                        u_all[:], sel[:, pb, :, :], pacc[:],
                        start=(pb == 0), stop=(pb == NB - 1),
                        perf_mode=PM.DoubleRow,
                    )

                # ---------- relu-accumulate chains (Scalar + DVE) -----------
                acc = accp.tile([128, nrow, O_BLK], FP8, tag="acc")
                for r in range(nrow):
                    nc.scalar.activation(acc[:, r, :], stks[2 * r][:], AF.Relu)
                    nc.vector.scalar_tensor_tensor(
                        acc[:, r, :], stks[2 * r + 1][:], 0.0, acc[:, r, :],
                        op0=ALU.max, op1=ALU.add,
                    )
                pend = (b, acc)

                # ---------- tail experts, once, early ----------
                if n_tail and b == tail_blk:
                    for e in range(n_tail):
                        tacc = rp.tile([128, OC], BF16, tag=f"tacc{e}")
                        nc.vector.scalar_tensor_tensor(
                            tacc[:], tbf[:, e, 0, :], h2[:, ns + e, 0:1],
                            tbf[:, e, 3, :], op0=ALU.mult, op1=ALU.add,
                        )
                        for i in (1, 2):
                            nc.vector.scalar_tensor_tensor(
                                tacc[:], tbf[:, e, i, :], h2[:, ns + e, i : i + 1],
                                tacc[:], op0=ALU.mult, op1=ALU.add,
                            )
                        if e == 0:
                            nc.scalar.activation(ut[:], tacc[:], AF.Relu)
                        else:
                            rt = rp.tile([128, OC], F32, tag="rt")
                            nc.scalar.activation(rt[:], tacc[:], AF.Relu)
                            nc.vector.tensor_tensor(ut[:], ut[:], rt[:], op=ALU.add)
                    nc.gpsimd.dma_start(d["ut_out"][:], ut[:])

            # ---------- final reduce + single u store ----------
            pb, pacc = pend
            nc.tensor.matmul(
                u_all[:], sel[:, pb, :, :], pacc[:],
                start=(pb == 0), stop=True, perf_mode=PM.DoubleRow,
            )
            u_sb = outp.tile([NB, O_BLK], F32, tag="u_sb")
            nc.vector.tensor_copy(u_sb[:], u_all[:])
            nc.sync.dma_start(d["u_out"][:], u_sb[:])

    nc.compile()
    return nc, d


def route(inputs):
    """Host-side routing: active experts + active dense columns."""
    x = np.asarray(inputs["x"], dtype=np.float32)
    sat = np.asarray(inputs["saturated"]).astype(bool)
    act = np.nonzero(sat & (x != 0))[0]
    dcols = np.nonzero(~sat)[0]
    per = -(-len(act) // N_CORES)            # ceil
    nslab = per // 128                       # full 128-expert slabs
    if nslab % 2:                            # DR reduce pairs slabs
        nslab -= 1
    n_tail = per - 128 * nslab
    nkc2 = -(-len(dcols) // 256)
    if nkc2 % 2:
        nkc2 += 1                            # dense pair-tiles need even kc
    return act, dcols, per, 0, nslab, n_tail, nkc2


def make_in_maps(inputs, act, dcols, per, nsub, nslab, n_tail, nkc2):
    x = np.asarray(inputs["x"], dtype=np.float32)
    weight = np.asarray(inputs["weight"], dtype=np.float32)
    bias = np.asarray(inputs["bias"], dtype=np.float32)
    W1 = np.asarray(inputs["W1"], dtype=np.float32)
    b1 = np.asarray(inputs["b1"], dtype=np.float32)
    W2 = np.asarray(inputs["W2"], dtype=np.float32)
    b2 = np.asarray(inputs["b2"], dtype=np.float32)
    W3 = np.asarray(inputs["W3"], dtype=np.float32)
    b3 = np.asarray(inputs["b3"], dtype=np.float32)

    ns = nslab
    m_own = SIZE_OUT // N_CORES
    NB = SIZE_OUT // O_BLK
    npair = nkc2 // 2
    n_slab = 128 * ns
    Dp = nkc2 * 256

    W38 = W3.astype(NP_FP8)                  # [N, O, 3]
    b38 = b3.astype(NP_FP8)                  # [N, O]

    xg_full = np.zeros(Dp, dtype=np.float32)
    xg_full[: len(dcols)] = x[dcols]
    # DoubleRow pairs: partition p of chunk kc holds rows kc*256+2p, +1
    xg = np.ascontiguousarray(
        xg_full.reshape(nkc2, 128, 2).transpose(1, 2, 0)
    ).astype(NP_FP8)

    cind = (H2S * np.eye(128, dtype=np.float32)).astype(NP_FP8)

    in_maps = []
    for i in range(N_CORES):
        ids = act[i * per : (i + 1) * per]
        n_live = len(ids)
        if n_live < per:
            ids = np.concatenate([ids, np.zeros(per - n_live, dtype=ids.dtype)])
        gids = ids[:n_slab]
        tids = ids[n_slab:]

        # ---- contiguous per-o-block expert super-tiles -------------------
        G = np.empty((n_slab, SIZE_OUT, 4), dtype=NP_FP8)
        G[:, :, 0:3] = W38[gids]
        G[:, :, 3] = b38[gids]
        live = min(max(n_live, 0), n_slab)
        if live < n_slab:
            G[live:] = 0
        pe = np.ascontiguousarray(
            G.reshape(ns, 128, NB, O_BLK, 4).transpose(2, 1, 0, 4, 3)
        ).reshape(NB, 128, ns, 2, 2, O_BLK)

        # ---- merged small-MLP consts (slab groups + tail groups) ---------
        def grp(a, shp):
            main = a[gids].reshape((ns, 128) + shp).transpose(
                (1, 0) + tuple(range(2, 2 + len(shp))))
            if n_tail:
                tailb = np.broadcast_to(a[tids], (128, n_tail) + shp)
                return np.ascontiguousarray(
                    np.concatenate([main, tailb], axis=1))
            return np.ascontiguousarray(main)

        m = {
            "pe": pe,
            "xg": xg,
            "cind": cind,
            "bias": bias[i * m_own : (i + 1) * m_own].reshape(1, m_own),
            "xall": grp(x, ()),
            "w1": grp(W1, (3, 3)),
            "b1": grp(b1, (3,)),
            "w2": grp(W2, (3, 3)),
            "b2": grp(b2, (3,)),
        }

        slm = slice(i * m_own, (i + 1) * m_own)
        wtg = np.zeros((Dp, m_own), dtype=np.float32)
        wtg[: len(dcols)] = weight[slm][:, dcols].T * WT_SCALE
        m["wt"] = np.ascontiguousarray(
            wtg.astype(NP_FP8).reshape(npair, 2, 128, 2, m_own)
            .transpose(0, 2, 1, 3, 4)
        )

        if n_tail:
            nt_live = max(0, min(n_tail, n_live - n_slab))
            w3tt = np.ascontiguousarray(
                W3[tids]
                .transpose(0, 2, 1)
                .reshape(n_tail, 3, OC, 128)
                .transpose(3, 0, 1, 2)
            ).astype(NP_BF16)
            b3tt = np.ascontiguousarray(
                b3[tids].reshape(n_tail, OC, 128).transpose(2, 0, 1)
            ).astype(NP_BF16)
            if nt_live < n_tail:
                w3tt[:, nt_live:] = 0
                b3tt[:, nt_live:] = 0
            tbf = np.empty((128, n_tail, 4, OC), dtype=NP_BF16)
            tbf[:, :, 0:3, :] = w3tt
            tbf[:, :, 3, :] = b3tt
            m["tbf"] = tbf
        in_maps.append(m)
    return in_maps


def combine_outputs(results, names, n_tail):
    u = np.zeros(SIZE_OUT, dtype=np.float64)
    dense = []
    for res in results:
        u += res[names["u_out"].name].reshape(-1).astype(np.float64)
        if n_tail:
            ut = res[names["ut_out"].name].astype(np.float64)  # [128, OC]
            u += ut.T.reshape(-1)                              # o = c*128 + p
        dense.append(res[names["dense_out"].name].reshape(-1))
    out = np.concatenate(dense).astype(np.float64) + u
    return out.astype(np.float32)


_CACHE = {}
CONFIG = {}


def _get_program(nsub, nslab, n_tail, nkc2):
    key = (nsub, nslab, n_tail, nkc2, tuple(sorted(CONFIG.items())))
    if key not in _CACHE:
        _CACHE[key] = build_program(nslab, n_tail, nkc2, **CONFIG)
    return _CACHE[key]


def kernel(**inputs):
    act, dcols, per, nsub, nslab, n_tail, nkc2 = route(inputs)
    nc, names = _get_program(nsub, nslab, n_tail, nkc2)
    in_maps = make_in_maps(inputs, act, dcols, per, nsub, nslab, n_tail, nkc2)
    keyed = [{names[k].name: v for k, v in m.items()} for m in in_maps]
    res = run_bass_kernel_spmd(nc, keyed, core_ids=list(range(N_CORES)))
    return combine_outputs(res.results, names, n_tail)


# revision 11
# speedup vs baseline: 1.3281x; 1.1352x over previous
"""BuddingLayer Trainium2 kernel (8-core, routed + fp8, contiguous-stream).

Reference computation (N = size_in = 8192, O = size_out = 8192):
    mask  = (x * saturated) != 0                   # ~half the neurons
    h2    = tiny per-neuron MLP(x)                              [N,3]
    h3    = relu(sum_i W3[n,o,i] * h2[n,i] + b3[n,o])           [N,O]
    u[o]  = sum_n mask[n] * h3[n,o]
    out   = weight @ (x * ~saturated) + bias + u

Host-side routing gathers the ~4112 active experts (mask=1) and ~4080
active dense columns; all big streams are fp8e4 (tolerance 2e-2).
Per-core stream ~21 MB -> ~60 us DMA floor at 358 GB/s.

v2 design (from 113.5us baseline profile):
  * The baseline spent ~130us of aggregate issue-engine time on ~200
    small strided DMAs (512B packets) and ~25us of Tensor time on bf16
    ones-reduce matmuls.  Both are restructured away:
  * Expert stream is pre-tiled on host into per-o-block CONTIGUOUS
    super-tiles pe[b] = [128, ns, 2(pair), 2(row), 512] fp8 -> ONE
    1 MB DMA per block (16 total), 8 KB/partition contiguous.
  * All ns=4 slabs go through the PE DoubleRow path (2 matmuls/slab
    into stk_s).  relu is a 2-op chain per psum tile pair: Scalar
    ACTIVATE(relu) writes row r of a [128,2,512] fp8 acc tile for slab
    2r, DVE scalar_tensor_tensor max-add accumulates slab 2r+1.
  * One DoubleRow reduce matmul per block contracts acc against a
    0.125-valued block-column selector -> row b of a single [16,512]
    psum bank accumulated across all 16 blocks; one copy + one store
    at the end.  (0.125 = 1/H2S undoes the stationary's 8x scale.)
  * Dense matvec: wt pre-tiled into 8 contiguous pair-tiles
    [128, 2(kc), 2(row), 1024] fp8, DoubleRow with xg stationary.
  * Small MLP consts for the 4 slab groups + 2 tail experts are merged
    (nt=6) into single per-field DMAs.
"""

import sys

import numpy as np

_TRN = "/opt/trn_rl_repo"
if _TRN not in sys.path:
    sys.path.insert(0, _TRN)

import ml_dtypes

import concourse.bacc as bacc
import concourse.mybir as mybir
from concourse import tile
from concourse.bass_utils import run_bass_kernel_spmd

F32 = mybir.dt.float32
BF16 = mybir.dt.bfloat16
FP8 = mybir.dt.float8e4
AF = mybir.ActivationFunctionType
ALU = mybir.AluOpType
AX = mybir.AxisListType
PM = mybir.MatmulPerfMode

NP_BF16 = ml_dtypes.bfloat16
NP_FP8 = ml_dtypes.float8_e4m3

N_CORES = 8
SIZE_IN = 8192
SIZE_OUT = 8192
OC = SIZE_OUT // 128          # o-chunks for the tail layout
O_BLK = 512                   # o-block (one psum bank of f32)
WT_SCALE = 1024.0             # dense weights are subnormal in fp8; prescale
H2S = 8.0                     # h2 scale in the PE stationary (undone in reduce)


def build_program(
    ns,                        # 128-expert PE slabs per core (must be even)
    n_tail,                    # leftover experts per core (o-transposed path)
    nkc2,                      # 256-row DoubleRow chunks for dense (even)
    size_out=SIZE_OUT,
    n_cores=N_CORES,
    pe_bufs=16,
    wt_bufs=8,
    acc_bufs=3,
    tail_blk=2,
    enable_asserts=False,
):
    assert ns % 2 == 0 and ns >= 2
    assert nkc2 % 2 == 0
    m_own = size_out // n_cores
    NB = size_out // O_BLK
    npair = nkc2 // 2
    nrow = ns // 2
    nt_all = ns + n_tail       # merged small-MLP groups (slab + tail)

    nc = bacc.Bacc(
        "TRN2",
        target_bir_lowering=False,
        debug=False,
        enable_asserts=enable_asserts,
        num_devices=n_cores,
    )

    d = {}
    d["pe"] = nc.dram_tensor("pe", [NB, 128, ns, 2, 2, O_BLK], FP8,
                             kind="ExternalInput")
    d["wt"] = nc.dram_tensor("wt", [npair, 128, 2, 2, m_own], FP8,
                             kind="ExternalInput")
    # packed consts: fpk = cind | xg (fp8), cpk = x | w1 | b1 | w2 | b2 (f32)
    d["fpk"] = nc.dram_tensor("fpk", [128, 128 + 2 * nkc2], FP8,
                              kind="ExternalInput")
    d["cpk"] = nc.dram_tensor("cpk", [128, 25 * nt_all], F32,
                              kind="ExternalInput")
    d["bias"] = nc.dram_tensor("bias", [1, m_own], F32, kind="ExternalInput")
    if n_tail:
        d["tbf"] = nc.dram_tensor("tbf", [128, n_tail, 4, OC], BF16,
                                  kind="ExternalInput")
        d["ut_out"] = nc.dram_tensor("ut_out", [128, OC], F32,
                                     kind="ExternalOutput")
    d["u_out"] = nc.dram_tensor("u_out", [NB, O_BLK], F32, kind="ExternalOutput")
    d["dense_out"] = nc.dram_tensor("dense_out", [1, m_own], F32,
                                    kind="ExternalOutput")

    def small_mlp(cp, x_sb, w1, b1, w2, b2, nt, pfx):
        h0 = cp.tile([128, nt], F32, tag=f"{pfx}h0")
        nc.vector.tensor_scalar_mul(h0[:], x_sb[:], 1.0 / 3.0)
        rs1 = cp.tile([128, nt, 3], F32, tag=f"{pfx}rs1")
        nc.vector.tensor_reduce(rs1[:], w1[:], axis=AX.X, op=ALU.add)
        h1 = cp.tile([128, nt, 3], F32, tag=f"{pfx}h1")
        for t in range(nt):
            nc.vector.scalar_tensor_tensor(
                h1[:, t, :], rs1[:, t, :], h0[:, t : t + 1], b1[:, t, :],
                op0=ALU.mult, op1=ALU.add,
            )
        nc.vector.tensor_scalar_max(h1[:], h1[:], 0.0)
        h2 = cp.tile([128, nt, 3], F32, tag=f"{pfx}h2")
        for t in range(nt):
            nc.vector.scalar_tensor_tensor(
                h2[:, t, :], w2[:, t, :, 0], h1[:, t, 0:1], b2[:, t, :],
                op0=ALU.mult, op1=ALU.add,
            )
            for i in (1, 2):
                nc.vector.scalar_tensor_tensor(
                    h2[:, t, :], w2[:, t, :, i], h1[:, t, i : i + 1], h2[:, t, :],
                    op0=ALU.mult, op1=ALU.add,
                )
        nc.vector.tensor_scalar_max(h2[:], h2[:], 0.0)
        return h2

    with tile.TileContext(nc) as tc:
        with (
            tc.tile_pool(name="const", bufs=1) as cp,
            tc.tile_pool(name="pep", bufs=pe_bufs) as pep,
            tc.tile_pool(name="wtp", bufs=wt_bufs) as wtp,
            tc.tile_pool(name="accp", bufs=acc_bufs) as accp,
            tc.tile_pool(name="rp", bufs=2) as rp,
            tc.tile_pool(name="outp", bufs=2) as outp,
            tc.tile_pool(name="pp", bufs=1, space="PSUM") as pp,
        ):
            # ---- packed const loads on scalar (HW DGE, runs before relus) --
            nt = nt_all
            fpk = cp.tile([128, 128 + 2 * nkc2], FP8)
            nc.scalar.dma_start(fpk[:], d["fpk"][:])
            cpk = cp.tile([128, 25 * nt], F32)
            nc.scalar.dma_start(cpk[:], d["cpk"][:])
            bias_sb = cp.tile([1, m_own], F32)
            nc.scalar.dma_start(bias_sb[:], d["bias"][:])
            if n_tail:
                tbf = cp.tile([128, n_tail, 4, OC], BF16)
                nc.scalar.dma_start(tbf[:], d["tbf"][:])
                ut = cp.tile([128, OC], F32)
            cind = fpk[:, 0:128]
            xg = fpk[:, 128 : 128 + 2 * nkc2].rearrange(
                "p (r k) -> p r k", r=2, k=nkc2)
            x_all = cpk[:, 0:nt]
            w1 = cpk[:, nt : 10 * nt].rearrange(
                "p (t a b) -> p t a b", t=nt, a=3, b=3)
            b1 = cpk[:, 10 * nt : 13 * nt].rearrange(
                "p (t a) -> p t a", t=nt, a=3)
            w2 = cpk[:, 13 * nt : 22 * nt].rearrange(
                "p (t a b) -> p t a b", t=nt, a=3, b=3)
            b2 = cpk[:, 22 * nt : 25 * nt].rearrange(
                "p (t a) -> p t a", t=nt, a=3)

            # ---- streaming DMA issues, all up-front ------------------------
            # Split the 1MB expert super-tiles across TWO dma queues (sync
            # evens, gpsimd odds): one hwdge queue tops out ~250 GB/s, two
            # sustain the ~400 GB/s the stream needs.  Neither engine has
            # compute, so issues run ahead, gated only by buffer recycling.
            # Dense wt pair-tiles split between the queues to balance bytes.
            pets = [pep.tile([128, ns, 2, 2, O_BLK], FP8, tag="pe",
                             name=f"pet{b}")
                    for b in range(NB)]
            wtts = [wtp.tile([128, 2, 2, m_own], FP8, tag="wt",
                             name=f"wtt{j}")
                    for j in range(npair)]
            half = npair // 2
            sync_seq = []
            gps_seq = []
            for b in range(NB):
                if b % 2 == 0:
                    sync_seq.append(("pe", b))
                    j = half + b // 2 - 1          # wt4..7 after pe4,6,8,10
                    if half <= j < npair:
                        sync_seq.append(("wt", j))
                else:
                    gps_seq.append(("pe", b))
                    j = b // 2                     # wt0..3 after pe1,3,5,7
                    if j < half:
                        gps_seq.append(("wt", j))
            for eng, seq in ((nc.sync, sync_seq), (nc.gpsimd, gps_seq)):
                for kind, idx in seq:
                    if kind == "pe":
                        eng.dma_start(pets[idx][:], d["pe"][idx : idx + 1])
                    else:
                        eng.dma_start(wtts[idx][:], d["wt"][idx : idx + 1])

            # ---- reduce selector stationaries (device-built) --------------
            # sel[:, b, r, j] = 0.125 iff j == b : directs block b's 2-row
            # relu-acc contraction into row b of the u psum bank.
            sel = cp.tile([128, NB, nrow, NB], FP8)
            nc.vector.memset(sel[:], 0.0)
            for b in range(NB):
                nc.vector.memset(sel[:, b, :, b : b + 1], 1.0 / H2S)

            # ---- h2 for all experts (slab groups + tail groups) -----------
            h2 = small_mlp(cp, x_all, w1, b1, w2, b2, nt_all, "v")

            # diagonal stationaries: Sa = (8I)*h2_0 | (8I)*h2_1,
            # Sb = (8I)*h2_2 | 8I   (bias row coefficient)
            stat = []
            for s in range(ns):
                Sa = cp.tile([128, 2, 128], FP8, tag=f"Sa{s}")
                Sb = cp.tile([128, 2, 128], FP8, tag=f"Sb{s}")
                for c in (0, 1):
                    nc.vector.tensor_scalar(
                        Sa[:, c, :], cind[:], h2[:, s, c : c + 1], None,
                        op0=ALU.mult,
                    )
                nc.vector.tensor_scalar(
                    Sb[:, 0, :], cind[:], h2[:, s, 2:3], None, op0=ALU.mult,
                )
                nc.vector.tensor_copy(Sb[:, 1, :], cind[:])
                stat.append((Sa, Sb))

            # ---- persistent psum tiles ------------------------------------
            u_all = pp.tile([NB, O_BLK], F32, tag="uall")
            d_psum = pp.tile([1, m_own], F32, tag="dpsum")

            # ---- main streamed loop ---------------------------------------
            pend = None            # (block, acc) awaiting its reduce matmul
            for b in range(NB):
                pet = pets[b]
                # ---------- PE path: 2 DoubleRow matmuls per slab -----------
                stks = []
                for s in range(ns):
                    stk = pp.tile([128, O_BLK], F32, tag=f"stk{s}")
                    Sa, Sb = stat[s]
                    nc.tensor.matmul(
                        stk[:], Sa[:], pet[:, s, 0, :, :],
                        start=True, stop=False, perf_mode=PM.DoubleRow,
                    )
                    nc.tensor.matmul(
                        stk[:], Sb[:], pet[:, s, 1, :, :],
                        start=False, stop=True, perf_mode=PM.DoubleRow,
                    )
                    stks.append(stk)

                # ---------- dense matvec pair-tile (DoubleRow) --------------
                if b < npair:
                    wtt = wtts[b]
                    for q in (0, 1):
                        kc = 2 * b + q
                        for mb in range(m_own // 512):
                            lo, hi = mb * 512, (mb + 1) * 512
                            nc.tensor.matmul(
                                d_psum[0:1, lo:hi],
                                xg[:, :, kc : kc + 1],
                                wtt[:, q, :, lo:hi],
                                start=(kc == 0), stop=(kc == nkc2 - 1),
                                perf_mode=PM.DoubleRow,
                            )
                    if b == npair - 1:
                        dense_sb = outp.tile([1, m_own], F32, tag="dense_sb")
                        nc.vector.scalar_tensor_tensor(
                            dense_sb[:], d_psum[:], 1.0 / WT_SCALE, bias_sb[:],
                            op0=ALU.mult, op1=ALU.add,
                        )
                        nc.gpsimd.dma_start(d["dense_out"][:], dense_sb[:])

                # ---------- deferred reduce for the previous block ----------
                if pend is not None:
                    pb, pacc = pend
                    nc.tensor.matmul(
                        u_all[:], sel[:, pb, :, :], pacc[:],
                        start=(pb == 0), stop=(pb == NB - 1),
                        perf_mode=PM.DoubleRow,
                    )

                # ---------- relu-accumulate chains (Scalar + DVE) -----------
                acc = accp.tile([128, nrow, O_BLK], FP8, tag="acc")
                for r in range(nrow):
                    nc.scalar.activation(acc[:, r, :], stks[2 * r][:], AF.Relu)
                    nc.vector.scalar_tensor_tensor(
                        acc[:, r, :], stks[2 * r + 1][:], 0.0, acc[:, r, :],
                        op0=ALU.max, op1=ALU.add,
                    )
                pend = (b, acc)

                # ---------- tail experts, once, early ----------
                if n_tail and b == tail_blk:
                    for e in range(n_tail):
                        tacc = rp.tile([128, OC], BF16, tag=f"tacc{e}")
                        nc.vector.scalar_tensor_tensor(
                            tacc[:], tbf[:, e, 0, :], h2[:, ns + e, 0:1],
                            tbf[:, e, 3, :], op0=ALU.mult, op1=ALU.add,
                        )
                        for i in (1, 2):
                            nc.vector.scalar_tensor_tensor(
                                tacc[:], tbf[:, e, i, :], h2[:, ns + e, i : i + 1],
                                tacc[:], op0=ALU.mult, op1=ALU.add,
                            )
                        if e == 0:
                            nc.scalar.activation(ut[:], tacc[:], AF.Relu)
                        else:
                            rt = rp.tile([128, OC], F32, tag="rt")
                            nc.scalar.activation(rt[:], tacc[:], AF.Relu)
                            nc.vector.tensor_tensor(ut[:], ut[:], rt[:], op=ALU.add)
                    nc.gpsimd.dma_start(d["ut_out"][:], ut[:])

            # ---------- final reduce + single u store ----------
            pb, pacc = pend
            nc.tensor.matmul(
                u_all[:], sel[:, pb, :, :], pacc[:],
                start=(pb == 0), stop=True, perf_mode=PM.DoubleRow,
            )
            u_sb = outp.tile([NB, O_BLK], F32, tag="u_sb")
            nc.vector.tensor_copy(u_sb[:], u_all[:])
            nc.sync.dma_start(d["u_out"][:], u_sb[:])

    nc.compile()
    return nc, d


def route(inputs):
    """Host-side routing: active experts + active dense columns."""
    x = np.asarray(inputs["x"], dtype=np.float32)
    sat = np.asarray(inputs["saturated"]).astype(bool)
    act = np.nonzero(sat & (x != 0))[0]
    dcols = np.nonzero(~sat)[0]
    per = -(-len(act) // N_CORES)            # ceil
    nslab = per // 128                       # full 128-expert slabs
    if nslab % 2:                            # DR reduce pairs slabs
        nslab -= 1
    n_tail = per - 128 * nslab
    nkc2 = -(-len(dcols) // 256)
    if nkc2 % 2:
        nkc2 += 1                            # dense pair-tiles need even kc
    return act, dcols, per, 0, nslab, n_tail, nkc2


def make_in_maps(inputs, act, dcols, per, nsub, nslab, n_tail, nkc2):
    x = np.asarray(inputs["x"], dtype=np.float32)
    weight = np.asarray(inputs["weight"], dtype=np.float32)
    bias = np.asarray(inputs["bias"], dtype=np.float32)
    W1 = np.asarray(inputs["W1"], dtype=np.float32)
    b1 = np.asarray(inputs["b1"], dtype=np.float32)
    W2 = np.asarray(inputs["W2"], dtype=np.float32)
    b2 = np.asarray(inputs["b2"], dtype=np.float32)
    W3 = np.asarray(inputs["W3"], dtype=np.float32)
    b3 = np.asarray(inputs["b3"], dtype=np.float32)

    ns = nslab
    m_own = SIZE_OUT // N_CORES
    NB = SIZE_OUT // O_BLK
    npair = nkc2 // 2
    n_slab = 128 * ns
    Dp = nkc2 * 256

    W38 = W3.astype(NP_FP8)                  # [N, O, 3]
    b38 = b3.astype(NP_FP8)                  # [N, O]

    xg_full = np.zeros(Dp, dtype=np.float32)
    xg_full[: len(dcols)] = x[dcols]
    # DoubleRow pairs: partition p of chunk kc holds rows kc*256+2p, +1
    xg = np.ascontiguousarray(
        xg_full.reshape(nkc2, 128, 2).transpose(1, 2, 0)
    ).astype(NP_FP8)

    cind = (H2S * np.eye(128, dtype=np.float32)).astype(NP_FP8)

    in_maps = []
    for i in range(N_CORES):
        ids = act[i * per : (i + 1) * per]
        n_live = len(ids)
        if n_live < per:
            ids = np.concatenate([ids, np.zeros(per - n_live, dtype=ids.dtype)])
        gids = ids[:n_slab]
        tids = ids[n_slab:]

        # ---- contiguous per-o-block expert super-tiles -------------------
        G = np.empty((n_slab, SIZE_OUT, 4), dtype=NP_FP8)
        G[:, :, 0:3] = W38[gids]
        G[:, :, 3] = b38[gids]
        live = min(max(n_live, 0), n_slab)
        if live < n_slab:
            G[live:] = 0
        pe = np.ascontiguousarray(
            G.reshape(ns, 128, NB, O_BLK, 4).transpose(2, 1, 0, 4, 3)
        ).reshape(NB, 128, ns, 2, 2, O_BLK)

        # ---- merged small-MLP consts (slab groups + tail groups) ---------
        def grp(a, shp):
            main = a[gids].reshape((ns, 128) + shp).transpose(
                (1, 0) + tuple(range(2, 2 + len(shp))))
            if n_tail:
                tailb = np.broadcast_to(a[tids], (128, n_tail) + shp)
                main = np.concatenate([main, tailb], axis=1)
            return main.reshape(128, -1)

        nt = ns + n_tail
        cpkarr = np.ascontiguousarray(np.concatenate(
            [grp(x, ()), grp(W1, (3, 3)), grp(b1, (3,)),
             grp(W2, (3, 3)), grp(b2, (3,))], axis=1, dtype=np.float32))
        fpkarr = np.empty((128, 128 + 2 * nkc2), dtype=NP_FP8)
        fpkarr[:, 0:128] = cind
        fpkarr[:, 128:] = xg.reshape(128, 2 * nkc2)

        m = {
            "pe": pe,
            "fpk": fpkarr,
            "cpk": cpkarr,
            "bias": bias[i * m_own : (i + 1) * m_own].reshape(1, m_own),
        }

        slm = slice(i * m_own, (i + 1) * m_own)
        wtg = np.zeros((Dp, m_own), dtype=np.float32)
        wtg[: len(dcols)] = weight[slm][:, dcols].T * WT_SCALE
        m["wt"] = np.ascontiguousarray(
            wtg.astype(NP_FP8).reshape(npair, 2, 128, 2, m_own)
            .transpose(0, 2, 1, 3, 4)
        )

        if n_tail:
            nt_live = max(0, min(n_tail, n_live - n_slab))
            w3tt = np.ascontiguousarray(
                W3[tids]
                .transpose(0, 2, 1)
                .reshape(n_tail, 3, OC, 128)
                .transpose(3, 0, 1, 2)
            ).astype(NP_BF16)
            b3tt = np.ascontiguousarray(
                b3[tids].reshape(n_tail, OC, 128).transpose(2, 0, 1)
            ).astype(NP_BF16)
            if nt_live < n_tail:
                w3tt[:, nt_live:] = 0
                b3tt[:, nt_live:] = 0
            tbf = np.empty((128, n_tail, 4, OC), dtype=NP_BF16)
            tbf[:, :, 0:3, :] = w3tt
            tbf[:, :, 3, :] = b3tt
            m["tbf"] = tbf
        in_maps.append(m)
    return in_maps


def combine_outputs(results, names, n_tail):
    u = np.zeros(SIZE_OUT, dtype=np.float64)
    dense = []
    for res in results:
        u += res[names["u_out"].name].reshape(-1).astype(np.float64)
        if n_tail:
            ut = res[names["ut_out"].name].astype(np.float64)  # [128, OC]
            u += ut.T.reshape(-1)                              # o = c*128 + p
        dense.append(res[names["dense_out"].name].reshape(-1))
    out = np.concatenate(dense).astype(np.float64) + u
    return out.astype(np.float32)


_CACHE = {}
CONFIG = {}


def _get_program(nsub, nslab, n_tail, nkc2):
    key = (nsub, nslab, n_tail, nkc2, tuple(sorted(CONFIG.items())))
    if key not in _CACHE:
        _CACHE[key] = build_program(nslab, n_tail, nkc2, **CONFIG)
    return _CACHE[key]


def kernel(**inputs):
    act, dcols, per, nsub, nslab, n_tail, nkc2 = route(inputs)
    nc, names = _get_program(nsub, nslab, n_tail, nkc2)
    in_maps = make_in_maps(inputs, act, dcols, per, nsub, nslab, n_tail, nkc2)
    keyed = [{names[k].name: v for k, v in m.items()} for m in in_maps]
    res = run_bass_kernel_spmd(nc, keyed, core_ids=list(range(N_CORES)))
    return combine_outputs(res.results, names, n_tail)
